# revision 15
# baseline (speedup 1.0000x reference)
"""CrossFusion block on 8 TRN2 NeuronCores.

Data-parallel over batch: 64 batches -> 8 cores x 8 batches.
Feature-major activations (feature dim on SBUF partitions); all layout
transposition host-side.

v3: all linear layers run fp8(e4m3) matmuls with hi/lo split-precision
compensation; pairs of 128-deep contraction tiles go through DoubleRow
matmuls (2 k-tiles per pass). The proj layer additionally compensates
the activation side (llm_embed split hi/lo host-side, 3-product chain)
because its error feeds the residual stream directly. LayerNorm
gains/biases are folded into the consuming weights host-side, so
on-device LN is xn = (x - mean) * inv_std only.

Activation-table discipline: stage P (proj + both input LNs + k/v) uses
only Sqrt/Copy/Identity; stage M (attention) only Exp; stage F (FFN)
Sqrt tails then Gelu -> 3 table loads total. LN2 stat matmuls run
inside stage M (squares on Pool, sums on PE, mean/var via DVE from
PSUM) but their Sqrt tails are deferred to stage F via a variance row.

Attention packs head pairs into shared PSUM banks (head 2p in columns
[0,256), head 2p+1 in [256,512)) so each softmax ACT/DVE op covers two
heads in one instruction. q-proj for batch b+1 is emitted between the
rowsum and broadcast matmuls of batch b to keep the PE busy during
softmax ACT/DVE latency.
"""

import sys

sys.path.insert(0, "/opt/trn_rl_repo")

import numpy as np
import ml_dtypes

import concourse.bass as bass
import concourse.tile as tile
from concourse import bacc, mybir
from concourse import bass_utils

BF16 = ml_dtypes.bfloat16
F8NP = ml_dtypes.float8_e4m3

B, LC, LL, LLAMA_DIM, DIM, HEADS = 64, 77, 256, 2048, 768, 8
HEAD_DIM = DIM // HEADS          # 96
SCALE = HEAD_DIM ** -0.5
FF = 4 * DIM                     # 3072
NCORES = 8
BPC = B // NCORES                # batches per core = 8
TQ = BPC * LL                    # llm tokens per core = 2048
TK = BPC * LC                    # clip tokens per core = 616
KT_D = DIM // 128                # 6
KT_L = LLAMA_DIM // 128          # 16
KT_F = FF // 128                 # 24
EPS = 1e-5
SW = 256.0                       # fp8 weight scale
SE = 16.0                        # fp8 llm_embed scale

F32 = mybir.dt.float32
BF = mybir.dt.bfloat16
F8 = mybir.dt.float8e4
AF = mybir.ActivationFunctionType
OP = mybir.AluOpType
DR = mybir.MatmulPerfMode.DoubleRow

# packed param tile column offsets (all f32, [128, PP_COLS])
PP_PROJB = 0          # 6
PP_F1B = 6            # 24
PP_KB = 30            # 8 cols (rows 0..95)
PP_EPS = 38
PP_COLS = 39


def build_nc():
    nc = bacc.Bacc("TRN2", target_bir_lowering=False, debug=False)

    embH = nc.dram_tensor("embH", (KT_L, 128, TQ), F8, kind="ExternalInput")
    embL = nc.dram_tensor("embL", (KT_L, 128, TQ), F8, kind="ExternalInput")
    clipT = nc.dram_tensor("clipT", (KT_D, 128, TK), BF, kind="ExternalInput")
    wpH = nc.dram_tensor("wpH", (KT_L, 128, DIM), F8, kind="ExternalInput")
    wpL = nc.dram_tensor("wpL", (KT_L, 128, DIM), F8, kind="ExternalInput")
    wqH = nc.dram_tensor("wqH", (KT_D, 128, DIM), F8, kind="ExternalInput")
    wqL = nc.dram_tensor("wqL", (KT_D, 128, DIM), F8, kind="ExternalInput")
    wkH = nc.dram_tensor("wkH", (KT_D, 128, DIM), F8, kind="ExternalInput")
    wkL = nc.dram_tensor("wkL", (KT_D, 128, DIM), F8, kind="ExternalInput")
    wvH = nc.dram_tensor("wvH", (KT_D, 128, DIM), F8, kind="ExternalInput")
    wvL = nc.dram_tensor("wvL", (KT_D, 128, DIM), F8, kind="ExternalInput")
    woH = nc.dram_tensor("woH", (HEAD_DIM, HEADS, DIM), F8, kind="ExternalInput")
    woL = nc.dram_tensor("woL", (HEAD_DIM, HEADS, DIM), F8, kind="ExternalInput")
    f1H = nc.dram_tensor("f1H", (KT_D, 128, FF), F8, kind="ExternalInput")
    f1L = nc.dram_tensor("f1L", (KT_D, 128, FF), F8, kind="ExternalInput")
    f2H = nc.dram_tensor("f2H", (KT_F, 128, DIM), F8, kind="ExternalInput")
    f2L = nc.dram_tensor("f2L", (KT_F, 128, DIM), F8, kind="ExternalInput")
    pp = nc.dram_tensor("pp", (128, PP_COLS), F32, kind="ExternalInput")
    # bf16 row vectors (already *SW): vb2, ob, f2b, qb
    rowv = nc.dram_tensor("rowv", (4, DIM), BF, kind="ExternalInput")
    outT = nc.dram_tensor("outT", (KT_D, 128, TQ), F32, kind="ExternalOutput")

    with tile.TileContext(nc) as tc:
        from contextlib import ExitStack
        with ExitStack() as stk:
            pw = stk.enter_context(tc.tile_pool(name="pw", bufs=1))
            pact = stk.enter_context(tc.tile_pool(name="pact", bufs=1))
            prow = stk.enter_context(tc.tile_pool(name="prow", bufs=1))
            prowb = stk.enter_context(tc.tile_pool(name="prowb", bufs=2))
            ptmp = stk.enter_context(tc.tile_pool(name="ptmp", bufs=2))
            psum = stk.enter_context(
                tc.tile_pool(name="psum", bufs=8, space="PSUM"))

            ones_sq = pw.tile([128, 128], BF, tag="ones")
            nc.vector.memset(ones_sq, 1.0)
            ones128 = ones_sq[:, 0:1]
            ones1 = ones_sq[0:1, :]
            onesrow = pw.tile([1, 512], BF, tag="onesrow")
            nc.vector.memset(onesrow, 1.0)
            pp_sb = pw.tile([128, PP_COLS], F32, tag="pp")
            nc.sync.dma_start(out=pp_sb, in_=pp.ap())
            vb_sb = pw.tile([1, DIM], BF, tag="vb")
            nc.sync.dma_start(out=vb_sb, in_=rowv.ap()[0:1])
            ob_sb = pw.tile([1, DIM], BF, tag="ob")
            nc.sync.dma_start(out=ob_sb, in_=rowv.ap()[1:2])
            f2b_sb = pw.tile([1, DIM], BF, tag="f2b")
            nc.sync.dma_start(out=f2b_sb, in_=rowv.ap()[2:3])
            qb_sb = pw.tile([1, DIM], BF, tag="qb")
            nc.sync.dma_start(out=qb_sb, in_=rowv.ap()[3:4])

            def ppc(col, n=1, rows=128):
                return pp_sb[:rows, col:col + n]

            def load3(pool, dram, shape, name):
                t = pool.tile(list(shape), dram.dtype, tag=name)
                for k in range(shape[1]):
                    nc.sync.dma_start(out=t[:, k, :], in_=dram.ap()[k])
                return t

            # ---------- LN helpers (g/b folded into weights host-side) ----
            def stats_mm(x_sb, n_kt, c0, cw, m_dst, var_dst):
                """PE/Pool/DVE part of LN stats: mean + variance rows."""
                ps_s = psum.tile([128, 512], F32, tag="ps",
                                 name=f"ps_s{c0}_{x_sb.tensor.name}")
                ps_q = psum.tile([128, 512], F32, tag="ps",
                                 name=f"ps_q{c0}_{x_sb.tensor.name}")
                for kt in range(n_kt):
                    xs = x_sb[:, kt, c0:c0 + cw]
                    sq = ptmp.tile([128, 512], BF, tag="sq")
                    nc.gpsimd.tensor_tensor(out=sq[:, :cw], in0=xs, in1=xs,
                                            op=OP.mult)
                    nc.tensor.matmul(ps_s[:1, :cw], ones128, xs,
                                     start=(kt == 0), stop=(kt == n_kt - 1))
                    nc.tensor.matmul(ps_q[:1, :cw], ones128, sq[:, :cw],
                                     start=(kt == 0), stop=(kt == n_kt - 1))
                nc.vector.tensor_scalar(out=m_dst, in0=ps_s[:1, :cw],
                                        scalar1=1.0 / DIM, scalar2=None,
                                        op0=OP.mult)
                c = ptmp.tile([1, 512], BF, tag="cvar")
                nc.vector.tensor_tensor(out=c[:, :cw], in0=m_dst, in1=m_dst,
                                        op=OP.mult)
                nc.vector.scalar_tensor_tensor(
                    out=var_dst, in0=ps_q[:1, :cw], scalar=1.0 / DIM,
                    in1=c[:, :cw], op0=OP.mult, op1=OP.subtract)

            def stats_tail(var_src, a_dst):
                """ACT Sqrt + DVE reciprocal: a_dst <- 1/sqrt(var+eps)."""
                nc.scalar.activation(out=var_src, in_=var_src, func=AF.Sqrt,
                                     bias=pp_sb[:1, PP_EPS:PP_EPS + 1])
                with nc.allow_low_precision("ln inv_std bf16"):
                    nc.vector.reciprocal(out=a_dst, in_=var_src)

            def ln_stats(x_sb, n_kt, c0, cw, a_dst, m_dst):
                v = ptmp.tile([1, 512], BF, tag="vrow")
                stats_mm(x_sb, n_kt, c0, cw, m_dst, v[:, :cw])
                stats_tail(v[:, :cw], a_dst)

            def ln_norm_chunk(x_view, out_view, n_kt, cw, a_src, m_src,
                              name=""):
                """out = (x - mean) * inv_std, fp8 out. a/m are [1,cw] APs."""
                ps_m = psum.tile([128, 512], F32, tag="ps", name=f"ps_m{name}")
                ps_i = psum.tile([128, 512], F32, tag="ps", name=f"ps_i{name}")
                nc.tensor.matmul(ps_m[:, :cw], ones1, m_src)
                nc.tensor.matmul(ps_i[:, :cw], ones1, a_src)
                for kt in range(n_kt):
                    t = ptmp.tile([128, 512], BF, tag="normt")
                    nc.vector.tensor_tensor(out=t[:, :cw],
                                            in0=x_view[:, kt, :cw],
                                            in1=ps_m[:, :cw], op=OP.subtract)
                    nc.vector.tensor_tensor(out=out_view[:, kt, :cw],
                                            in0=t[:, :cw],
                                            in1=ps_i[:, :cw], op=OP.mult)

            # ---------- persistent activations ----------
            llm_sb = pact.tile([128, KT_D, TQ], BF, tag="llm")
            llmn_sb = pact.tile([128, KT_D, TQ], F8, tag="llmn")
            k_sb = pact.tile([HEAD_DIM, HEADS, TK], F8, tag="k")
            v_sb = pact.tile([LC, BPC, DIM], F8, tag="v")

            a_2 = prow.tile([1, TQ], BF, tag="a_2")
            m_2 = prow.tile([1, TQ], BF, tag="m_2")
            v_2 = prow.tile([1, TQ], BF, tag="v_2")

            # ================= weights ====================================
            pqw = stk.enter_context(tc.tile_pool(name="pqw", bufs=1))
            pwproj = tc.alloc_tile_pool(name="pwproj", bufs=1)
            pemb = tc.alloc_tile_pool(name="pemb", bufs=2)

            embs = {}

            def emit_emb_dma(b):
                c0 = b * LL
                ehi = pemb.tile([128, KT_L, LL], F8, tag="ehi", name=f"ehi{b}")
                elo = pemb.tile([128, KT_L, LL], F8, tag="elo", name=f"elo{b}")
                for kt in range(KT_L):
                    nc.sync.dma_start(out=ehi[:, kt, :],
                                      in_=embH.ap()[kt, :, c0:c0 + LL])
                    nc.sync.dma_start(out=elo[:, kt, :],
                                      in_=embL.ap()[kt, :, c0:c0 + LL])
                embs[b] = (ehi, elo)

            # ====== stage P part 1: clip path (LN_q + k + v) ==============
            # DMA issue order == consumption order: clip first, then
            # emb/proj weights, then k/v weights, then q/o weights.
            with tc.tile_pool(name="pclip", bufs=1) as pclip, \
                 tc.tile_pool(name="pkvw", bufs=1) as pkvw:
                clip_sb = load3(pclip, clipT, (128, KT_D, TK), "clip")
                emit_emb_dma(0)
                emit_emb_dma(1)
                wph_sb = load3(pwproj, wpH, (128, KT_L, DIM), "wph")
                wpl_sb = load3(pwproj, wpL, (128, KT_L, DIM), "wpl")
                clipn_sb = pclip.tile([128, KT_D, TK], F8, tag="clipn")
                a_c = prow.tile([1, TK], BF, tag="a_c")
                m_c = prow.tile([1, TK], BF, tag="m_c")
                for ci in range(2):
                    c0 = ci * 308
                    ln_stats(clip_sb, KT_D, c0, 308,
                             a_c[:, c0:c0 + 308], m_c[:, c0:c0 + 308])
                for ci in range(2):
                    c0 = ci * 308
                    ln_norm_chunk(clip_sb[:, :, c0:c0 + 308],
                                  clipn_sb[:, :, c0:c0 + 308], KT_D, 308,
                                  a_c[:, c0:c0 + 308], m_c[:, c0:c0 + 308],
                                  name=f"cl{ci}")

                wkh_sb = load3(pkvw, wkH, (128, KT_D, DIM), "wkh")
                wkl_sb = load3(pkvw, wkL, (128, KT_D, DIM), "wkl")
                wvh_sb = load3(pkvw, wvH, (128, KT_D, DIM), "wvh")
                wvl_sb = load3(pkvw, wvL, (128, KT_D, DIM), "wvl")

                # k.T head-major [96, h, 616]
                for h in range(HEADS):
                    for ci in range(2):
                        c0 = ci * 308
                        ps = psum.tile([128, 512], F32, tag="ps",
                                       name=f"ps_k{h}_{ci}")
                        idx = 0
                        for wt in (wkh_sb, wkl_sb):
                            for t in range(KT_D // 2):
                                nc.tensor.matmul(
                                    ps[:HEAD_DIM, :308],
                                    wt[:, 2 * t:2 * t + 2, h * 96:(h + 1) * 96],
                                    clipn_sb[:, 2 * t:2 * t + 2, c0:c0 + 308],
                                    start=(idx == 0), stop=(idx == 5),
                                    perf_mode=DR)
                                idx += 1
                        nc.scalar.activation(
                            out=k_sb[:, h, c0:c0 + 308],
                            in_=ps[:HEAD_DIM, :308], func=AF.Identity,
                            scale=1.0 / SW, bias=ppc(PP_KB + h, rows=96))

                # v token-major [77, b, 768] (activation stationary: plain
                # fp8, M=77 violates dual-fp8 ldweights restrictions)
                for b in range(BPC):
                    for ci in range(2):
                        c0 = ci * 384
                        ps = psum.tile([128, 512], F32, tag="ps",
                                       name=f"ps_v{b}_{ci}")
                        idx = 0
                        for wt in (wvh_sb, wvl_sb):
                            for t in range(KT_D):
                                nc.tensor.matmul(
                                    ps[:LC, :384],
                                    clipn_sb[:, t, b * LC:(b + 1) * LC],
                                    wt[:, t, c0:c0 + 384],
                                    start=(idx == 0), stop=False)
                                idx += 1
                        nc.tensor.matmul(ps[:LC, :384], ones1[:, :LC],
                                         vb_sb[:, c0:c0 + 384],
                                         start=False, stop=True)
                        nc.scalar.activation(
                            out=v_sb[:, b, c0:c0 + 384], in_=ps[:LC, :384],
                            func=AF.Copy, scale=1.0 / SW)

            wqh_sb = load3(pqw, wqH, (128, KT_D, DIM), "wqh")
            wql_sb = load3(pqw, wqL, (128, KT_D, DIM), "wql")
            woh_sb = pqw.tile([HEAD_DIM, HEADS, DIM], F8, tag="woh")
            wol_sb = pqw.tile([HEAD_DIM, HEADS, DIM], F8, tag="wol")
            for h in range(HEADS):
                nc.sync.dma_start(out=woh_sb[:, h, :], in_=woH.ap()[:, h, :])
                nc.sync.dma_start(out=wol_sb[:, h, :], in_=woL.ap()[:, h, :])

            # f1 weights on the right heap side: DMA overlaps stage P/M
            pf1 = stk.enter_context(tc.tile_pool(name="pf1", bufs=1,
                                                 side="right"))
            f1h_sb = load3(pf1, f1H, (128, KT_D, FF), "f1h")
            f1l_sb = load3(pf1, f1L, (128, KT_D, FF), "f1l")
            pffn = stk.enter_context(tc.tile_pool(name="pffn", bufs=2,
                                                  side="right"))
            pfc = stk.enter_context(tc.tile_pool(name="pfc", bufs=1,
                                                 side="right"))
            pmid = stk.enter_context(tc.tile_pool(name="pmid", bufs=2,
                                                  side="right"))
            patn = stk.enter_context(tc.tile_pool(name="patn", bufs=4,
                                                  side="right"))

            # ====== stage P part 2: proj + LN_kv + lnn (all batches) ======
            def emit_proj_mt(b, mt):
                """One proj output tile [128, 256] for batch b."""
                ehi, elo = embs[b]
                c0 = b * LL
                ps = psum.tile([128, 512], F32, tag="ps", name=f"ps_p{b}_{mt}")
                idx = 0
                for et, wt in ((ehi, wph_sb), (ehi, wpl_sb), (elo, wph_sb)):
                    for t in range(KT_L // 2):
                        nc.tensor.matmul(
                            ps[:, :LL],
                            wt[:, 2 * t:2 * t + 2, mt * 128:(mt + 1) * 128],
                            et[:, 2 * t:2 * t + 2, :],
                            start=(idx == 0), stop=(idx == 23), perf_mode=DR)
                        idx += 1
                nc.scalar.activation(
                    out=llm_sb[:, mt, c0:c0 + LL], in_=ps[:, :LL],
                    func=AF.Identity, scale=1.0 / (SW * SE),
                    bias=ppc(PP_PROJB + mt))
                if mt == KT_D - 1:
                    embs.pop(b)

            def emit_kvln(b):
                c0 = b * LL
                a_t = prowb.tile([1, LL], BF, tag="akv", name=f"akv{b}")
                m_t = prowb.tile([1, LL], BF, tag="mkv", name=f"mkv{b}")
                ln_stats(llm_sb, KT_D, c0, LL, a_t[0:1, :], m_t[0:1, :])
                ln_norm_chunk(llm_sb[:, :, c0:c0 + LL],
                              llmn_sb[:, :, c0:c0 + LL], KT_D, LL,
                              a_t[0:1, :], m_t[0:1, :], name=f"kv{b}")

            # stats/norm of batch b-1 emitted after proj of batch b so the
            # PE is never gated on the DVE/ACT stats tail (p-state!)
            for b in range(BPC):
                if b + 2 < BPC:
                    emit_emb_dma(b + 2)
                for mt in range(KT_D):
                    emit_proj_mt(b, mt)
                if b >= 1:
                    emit_kvln(b - 1)
            emit_kvln(BPC - 1)
            pemb.release()
            pwproj.release()

            # f2 weights: DMA overlaps stage M
            pf2 = stk.enter_context(tc.tile_pool(name="pf2", bufs=1))
            f2h_sb = load3(pf2, f2H, (128, KT_F, DIM), "f2h")
            f2l_sb = load3(pf2, f2L, (128, KT_F, DIM), "f2l")
            pout = stk.enter_context(tc.tile_pool(name="pout", bufs=2))

            # ====== stage M: attention (Exp only on ACT) ==================
            def emit_q_pair(b, p, t):
                ps = psum.tile([128, 512], F32, tag="ps",
                               name=f"ps_qp_{b}_{p}")
                for i, h in enumerate((2 * p, 2 * p + 1)):
                    co = i * LL
                    idx = 0
                    for wt in (wqh_sb, wql_sb):
                        for tt in range(KT_D // 2):
                            nc.tensor.matmul(
                                ps[:HEAD_DIM, co:co + LL],
                                wt[:, 2 * tt:2 * tt + 2,
                                   h * 96:(h + 1) * 96],
                                llmn_sb[:, 2 * tt:2 * tt + 2,
                                        b * LL:(b + 1) * LL],
                                start=(idx == 0 and i == 0), stop=False,
                                perf_mode=DR)
                            idx += 1
                    nc.tensor.matmul(
                        ps[:HEAD_DIM, co:co + LL],
                        qb_sb[:, h * 96:(h + 1) * 96], onesrow[:, :LL],
                        start=False, stop=(i == 1))
                nc.scalar.activation(
                    out=t[:, 2 * p:2 * p + 2, :], in_=ps[:HEAD_DIM, :],
                    func=AF.Copy, scale=1.0 / SW)

            def emit_o(b, ao_c, mts):
                c0 = b * LL
                for mt in mts:
                    ps = psum.tile([128, 512], F32, tag="ps",
                                   name=f"ps_o{b}_{mt}")
                    idx = 0
                    for wt in (woh_sb, wol_sb):
                        for hh in range(HEADS // 2):
                            nc.tensor.matmul(
                                ps[:, :LL],
                                wt[:, 2 * hh:2 * hh + 2,
                                   mt * 128:(mt + 1) * 128],
                                ao_c[:, 2 * hh:2 * hh + 2, :],
                                start=(idx == 0), stop=False, perf_mode=DR)
                            idx += 1
                    nc.tensor.matmul(ps[:, :LL],
                                     ob_sb[:, mt * 128:(mt + 1) * 128],
                                     onesrow[:, :LL], start=False, stop=True)
                    nc.vector.scalar_tensor_tensor(
                        out=llm_sb[:, mt, c0:c0 + LL], in0=ps[:, :LL],
                        scalar=1.0 / SW, in1=llm_sb[:, mt, c0:c0 + LL],
                        op0=OP.mult, op1=OP.add)

            q_cs = {}

            def emit_q(b, pairs):
                if b not in q_cs:
                    q_cs[b] = pmid.tile([HEAD_DIM, HEADS, LL], F8, tag="q_c",
                                        name=f"q_c{b}")
                for p in pairs:
                    emit_q_pair(b, p, q_cs[b])

            emit_q(0, range(4))
            prev = {}

            for b in range(BPC):
                q_c = q_cs.pop(b)
                ps1, ps2, psv, ex, inv = {}, {}, {}, {}, {}

                def sc(p):
                    ps1[p] = psum.tile([128, 512], F32, tag="ps",
                                       name=f"ps1_{b}_{p}")
                    for i, h in enumerate((2 * p, 2 * p + 1)):
                        nc.tensor.matmul(ps1[p][:LC, i * LL:(i + 1) * LL],
                                         k_sb[:, h, b * LC:(b + 1) * LC],
                                         q_c[:, h, :],
                                         start=(i == 0), stop=(i == 1))
                    ex[p] = patn.tile([LC, 2 * LL], F8, tag="ex",
                                      name=f"ex_{b}_{p}")
                    nc.scalar.activation(out=ex[p], in_=ps1[p][:LC, :],
                                         func=AF.Exp, scale=SCALE)

                def rs(p):
                    ps2[p] = psum.tile([128, 512], F32, tag="ps",
                                       name=f"ps2_{b}_{p}")
                    for i in range(2):
                        nc.tensor.matmul(ps2[p][:1, i * LL:(i + 1) * LL],
                                         ones128[:LC, :],
                                         ex[p][:, i * LL:(i + 1) * LL],
                                         start=(i == 0), stop=(i == 1))
                    inv[p] = patn.tile([1, 2 * LL], BF, tag="inv",
                                       name=f"inv_{b}_{p}")
                    with nc.allow_low_precision("softmax 1/sum bf16"):
                        nc.vector.reciprocal(out=inv[p], in_=ps2[p][:1, :])

                def bc(p):
                    for i in range(2):
                        nc.tensor.matmul(ps2[p][:LC, i * LL:(i + 1) * LL],
                                         ones1[:, :LC],
                                         inv[p][:, i * LL:(i + 1) * LL],
                                         start=(i == 0), stop=(i == 1))
                    nc.vector.tensor_tensor(out=ex[p], in0=ex[p],
                                            in1=ps2[p][:LC, :], op=OP.mult)

                ao_c = pmid.tile([HEAD_DIM, HEADS, LL], F8, tag="ao_c",
                                 name=f"ao_c{b}")

                def av(p):
                    psv[p] = psum.tile([128, 512], F32, tag="ps",
                                       name=f"psv_{b}_{p}")
                    for i, h in enumerate((2 * p, 2 * p + 1)):
                        nc.tensor.matmul(
                            psv[p][:HEAD_DIM, i * LL:(i + 1) * LL],
                            v_sb[:, b, h * 96:(h + 1) * 96],
                            ex[p][:, i * LL:(i + 1) * LL],
                            start=(i == 0), stop=(i == 1))
                    nc.scalar.activation(out=ao_c[:, 2 * p:2 * p + 2, :],
                                         in_=psv[p][:HEAD_DIM, :],
                                         func=AF.Copy)

                # software pipeline: batch b-1's o-proj / LN2 stats fill
                # the PE while batch b's softmax runs on ACT/DVE.
                for p in range(4):
                    sc(p)
                if prev:
                    emit_o(prev["b"], prev["ao"], range(3))
                for p in range(4):
                    rs(p)
                if prev:
                    emit_o(prev["b"], prev["ao"], range(3, KT_D))
                for p in range(4):
                    bc(p)
                if prev:
                    pc0 = prev["b"] * LL
                    stats_mm(llm_sb, KT_D, pc0, LL, m_2[:, pc0:pc0 + LL],
                             v_2[:, pc0:pc0 + LL])
                if b + 1 < BPC:
                    emit_q(b + 1, range(4))
                for p in range(4):
                    av(p)
                prev = {"b": b, "ao": ao_c}

            emit_o(prev["b"], prev["ao"], range(KT_D))
            c0 = prev["b"] * LL
            stats_mm(llm_sb, KT_D, c0, LL, m_2[:, c0:c0 + LL],
                     v_2[:, c0:c0 + LL])

            # ====== stage F: LN2 tails + FFN ==============================
            for c in range(4):
                c0 = c * 512
                stats_tail(v_2[:, c0:c0 + 512], a_2[:, c0:c0 + 512])

            h_cs = {}

            def emit_h(cc):
                t = pffn.tile([128, KT_D, 512], F8, tag="h_c", name=f"h_c{cc}")
                c0 = cc * 512
                ln_norm_chunk(llm_sb[:, :, c0:c0 + 512],
                              t, KT_D, 512, a_2[:, c0:c0 + 512],
                              m_2[:, c0:c0 + 512], name=f"n2{cc}")
                h_cs[cc] = t

            f_cs = {}

            def emit_f1_mt(cc, mt):
                if cc not in f_cs:
                    f_cs[cc] = pfc.tile([128, KT_F, 512], F8, tag="f_c",
                                        name=f"f_c{cc}")
                f_c = f_cs[cc]
                h_c = h_cs[cc]
                ps = psum.tile([128, 512], F32, tag="ps",
                               name=f"ps_f1_{cc}_{mt}")
                idx = 0
                for wt in (f1h_sb, f1l_sb):
                    for t in range(KT_D // 2):
                        nc.tensor.matmul(
                            ps, wt[:, 2 * t:2 * t + 2, mt * 128:(mt + 1) * 128],
                            h_c[:, 2 * t:2 * t + 2, :],
                            start=(idx == 0), stop=(idx == 5), perf_mode=DR)
                        idx += 1
                nc.scalar.activation(
                    out=f_c[:, mt, :], in_=ps, func=AF.Gelu_apprx_sigmoid,
                    scale=1.0 / SW, bias=ppc(PP_F1B + mt))

            NCH = 512
            NFC = TQ // NCH
            emit_h(0)
            for ci in range(NFC):
                for mt in range(KT_F):
                    emit_f1_mt(ci, mt)
                if ci + 1 < NFC:
                    emit_h(ci + 1)
                f_c = f_cs.pop(ci)
                h_cs.pop(ci)
                c0 = ci * NCH
                for mt in range(KT_D):
                    ps = psum.tile([128, 512], F32, tag="ps",
                                   name=f"ps_f2_{ci}_{mt}")
                    idx = 0
                    for wt in (f2h_sb, f2l_sb):
                        for t in range(KT_F // 2):
                            nc.tensor.matmul(
                                ps,
                                wt[:, 2 * t:2 * t + 2, mt * 128:(mt + 1) * 128],
                                f_c[:, 2 * t:2 * t + 2, :],
                                start=(idx == 0), stop=False, perf_mode=DR)
                            idx += 1
                    nc.tensor.matmul(ps, f2b_sb[:, mt * 128:(mt + 1) * 128],
                                     onesrow[:, :NCH], start=False, stop=True)
                    o_c = pout.tile([128, NCH], F32, tag="o_c")
                    nc.vector.scalar_tensor_tensor(
                        out=o_c, in0=ps, scalar=1.0 / SW,
                        in1=llm_sb[:, mt, c0:c0 + NCH],
                        op0=OP.mult, op1=OP.add)
                    nc.sync.dma_start(out=outT.ap()[mt, :, c0:c0 + NCH],
                                      in_=o_c)

    nc.compile()
    return nc


_NC_CACHE = {}


def _get_nc():
    if "nc" not in _NC_CACHE:
        _NC_CACHE["nc"] = build_nc()
    return _NC_CACHE["nc"]


def _hilo(w):
    """Split f32 array (already scaled) into fp8 hi/lo."""
    hi = w.astype(F8NP)
    lo = (w - hi.astype(np.float32)).astype(F8NP)
    return hi, lo


def _prep_in_maps(inputs):
    f32 = np.float32
    g = {k: np.asarray(v, f32) for k, v in inputs.items()}

    # fold LN gains/biases into consuming weights
    kw = g["k_w"] * g["qn_g"][None, :]
    kb = g["k_b"] + g["k_w"] @ g["qn_b"]
    vw = g["v_w"] * g["qn_g"][None, :]
    vb = g["v_b"] + g["v_w"] @ g["qn_b"]
    qw = g["q_w"] * g["kvn_g"][None, :]
    qb = g["q_b"] + g["q_w"] @ g["kvn_b"]
    f1w = g["f1_w"] * g["n_g"][None, :]
    f1b = g["f1_b"] + g["f1_w"] @ g["n_b"]

    w = {}

    def put_hl(name, arr):
        hi, lo = _hilo(arr * SW)
        w[name + "H"] = np.ascontiguousarray(hi)
        w[name + "L"] = np.ascontiguousarray(lo)

    put_hl("wp", g["llm_proj_w"].T.reshape(KT_L, 128, DIM))
    put_hl("wq", qw.T.reshape(KT_D, 128, DIM))
    put_hl("wk", kw.T.reshape(KT_D, 128, DIM))
    put_hl("wv", vw.T.reshape(KT_D, 128, DIM))
    put_hl("wo", np.ascontiguousarray(
        g["o_w"].T.reshape(HEADS, HEAD_DIM, DIM).transpose(1, 0, 2)))
    put_hl("f1", f1w.T.reshape(KT_D, 128, FF))
    put_hl("f2", g["f2_w"].T.reshape(KT_F, 128, DIM))

    rowv = np.zeros((4, DIM), f32)
    rowv[0] = vb * SW
    rowv[1] = g["o_b"] * SW
    rowv[2] = g["f2_b"] * SW
    rowv[3] = qb * SW
    w["rowv"] = rowv.astype(BF16)

    ppa = np.zeros((128, PP_COLS), dtype=f32)

    def put(col, vec, n):
        ppa[:, col:col + n] = np.asarray(vec, dtype=f32).reshape(n, 128).T

    put(PP_PROJB, g["llm_proj_b"], KT_D)
    put(PP_F1B, f1b, KT_F)
    ppa[:HEAD_DIM, PP_KB:PP_KB + HEADS] = kb.reshape(HEADS, HEAD_DIM).T
    ppa[:, PP_EPS] = EPS
    w["pp"] = ppa

    clip = g["clip_embed"]
    llm = g["llm_embed"]
    in_maps = []
    for c in range(NCORES):
        cs = slice(c * BPC, (c + 1) * BPC)
        m = dict(w)
        embT = llm[cs].reshape(TQ, LLAMA_DIM).T.reshape(KT_L, 128, TQ) * SE
        ehi, elo = _hilo(embT)
        m["embH"] = np.ascontiguousarray(ehi)
        m["embL"] = np.ascontiguousarray(elo)
        m["clipT"] = np.ascontiguousarray(
            clip[cs].reshape(TK, DIM).T.reshape(KT_D, 128, TK)).astype(BF16)
        in_maps.append(m)
    return in_maps


def run(inputs, trace=False):
    nc = _get_nc()
    in_maps = _prep_in_maps(inputs)
    res = bass_utils.run_bass_kernel_spmd(
        nc, in_maps, core_ids=list(range(NCORES)), trace=trace)
    clip = np.asarray(inputs["clip_embed"], dtype=np.float32)
    llm3 = np.empty((B, LL, DIM), dtype=np.float32)
    for c in range(NCORES):
        yT = res.results[c]["outT"].reshape(DIM, TQ)
        llm3[c * BPC:(c + 1) * BPC] = yT.T.reshape(BPC, LL, DIM)
    out = np.concatenate([clip, llm3], axis=1)
    return out, res


def kernel(**inputs):
    out, _ = run(inputs, trace=False)
    return out
